# revision 2
# baseline (speedup 1.0000x reference)
"""DifferentiableXGB forward on 8 TRN2 NeuronCores — v6.

Baseline (69us) compute, unchanged op-for-op; only the x data path is
restructured: tile-major DRAM layouts so each (tensor, batch-tile) is ONE
DMA of 128 contiguous runs (16 issues/rep instead of 64 — each dma_start
costs ~650ns serialized on SP+HWDGE).

Per-core device program (identical math to baseline):
  split_k^T [112t, b] = x8 . W8_k      (fp8 DoubleRow, 4 passes of 256)
  s^T [100t, b]       = xb . Ws        (bf16 exact path)
  leaf_k^T = sigmoid(split_k^T/256 + b1[:,k])   (ACT)
  prod_k^T = leaf_k^T * s^T                      (DVE, f32r out)
  y^T [2, b] = sum_k g_k^T prod_k^T + fc_b       (PE accum + ACT bias)
"""
import time
import numpy as np
from contextlib import ExitStack

N_CORES = 8
B, D, T, K = 32768, 1024, 100, 4
BL = B // N_CORES  # batch rows per core
NBT = BL // 512    # 512-wide batch tiles per core
ND = D // 128      # 128-deep contraction chunks
NC2 = D // 256     # 256-deep DoubleRow chunks
TP = 112           # tree channels padded (step%16==0 for DoubleRow lhsT)
WS = 256.0         # fp8 weight scale

_cache = {}


def build_pools(reps, pools):
    global _POOLS
    _POOLS = pools
    try:
        return build(reps)
    finally:
        _POOLS = (6, 1, 1)


_POOLS = (6, 1, 1)


def build(reps=0):
    """Build + compile the per-core Bass program. reps>0 wraps the compute
    body in a HW loop executing it `reps` times (for steady-state timing)."""
    from concourse import bacc
    import concourse.mybir as mybir
    import concourse.tile as tile

    f32, f32r = mybir.dt.float32, mybir.dt.float32r
    bf16, f8 = mybir.dt.bfloat16, mybir.dt.float8e4
    AF = mybir.ActivationFunctionType
    DR = mybir.MatmulPerfMode.DoubleRow

    nc = bacc.Bacc("TRN2", target_bir_lowering=False, debug=False)
    x8 = nc.dram_tensor("x8", [128, NBT, ND, 512], f8, kind="ExternalInput")
    xb = nc.dram_tensor("xb", [128, NBT, ND, 512], bf16, kind="ExternalInput")
    w8 = nc.dram_tensor("w8", [K * NC2, 128, 2, TP], f8, kind="ExternalInput")
    wsb = nc.dram_tensor("wsb", [ND, 128, T], bf16, kind="ExternalInput")
    # bias [0:8] | g 4 blocks of 32 [8:136] | fcb col 136 — one DMA: separate
    # small const DMAs were observed to corrupt unrelated SBUF loads
    consts = nc.dram_tensor("consts", [T, 140], f32r, kind="ExternalInput")
    y = nc.dram_tensor("y", [2, BL], f32, kind="ExternalOutput")

    with ExitStack() as ctx:
        tc = ctx.enter_context(tile.TileContext(nc))
        cp = ctx.enter_context(tc.tile_pool(name="const", bufs=1))
        wp = ctx.enter_context(tc.tile_pool(name="wp", bufs=1))
        xp = ctx.enter_context(tc.tile_pool(name="xp", bufs=1))
        ep = ctx.enter_context(tc.tile_pool(name="ep", bufs=3))
        # sp=6: split psum slots span two tiles of slack so a tile's matmuls
        # never serialize behind the previous tile's sigmoid reads
        sp = ctx.enter_context(tc.tile_pool(name="sp", bufs=_POOLS[0], space="PSUM"))
        ssp = ctx.enter_context(tc.tile_pool(name="ssp", bufs=_POOLS[1], space="PSUM"))
        op = ctx.enter_context(tc.tile_pool(name="op", bufs=_POOLS[2], space="PSUM"))

        consts_sb = cp.tile([T, 140], f32r, name="consts_sb", tag="consts_sb")
        nc.sync.dma_start(consts_sb[:], consts.ap())
        bias_sb = consts_sb[:, 0:8].bitcast(f32)
        g_sb = consts_sb[:, 8:136]
        fcb_sb = consts_sb[0:2, 136:137].bitcast(f32)
        out_sb = cp.tile([2, BL], f32, name="out_sb", tag="out_sb")

        w8s = []
        for i in range(K * NC2):
            wt = wp.tile([128, 2, TP], f8, name=f"w8_{i}", tag=f"w8_{i}")
            nc.sync.dma_start(wt[:], w8.ap()[i])
            w8s.append(wt)
        wss = []
        for d in range(ND):
            wt = wp.tile([128, T], bf16, name=f"ws{d}", tag=f"ws{d}")
            nc.sync.dma_start(wt[:], wsb.ap()[d])
            wss.append(wt)

        x8_sb = xp.tile([128, NBT, ND, 512], f8, name="x8_sb", tag="x8_sb")
        xb_sb = xp.tile([128, NBT * ND, 512], bf16, name="xb_sb", tag="xb_sb")

        x8ap, xbap = x8.ap(), xb.ap()

        def emit_body():
            # one DMA per (tensor, tile): 128 partitions x 4/8KB contiguous
            for bt in range(NBT):
                nc.sync.dma_start(x8_sb[:, bt], x8ap[:, bt])
                nc.sync.dma_start(xb_sb[:, bt * ND : (bt + 1) * ND, :], xbap[:, bt])
            for bt in range(NBT):
                sl = slice(bt * 512, (bt + 1) * 512)
                # exact-s group, bf16 over 8 chunks of 128
                pss = ssp.tile([T, 512], f32, name=f"ss{bt}", tag="ss")
                for d in range(ND):
                    nc.tensor.matmul(
                        pss[:],
                        wss[d][:],
                        xb_sb[:, bt * ND + d, :],
                        start=(d == 0),
                        stop=(d == ND - 1),
                    )
                # 4 split groups, fp8 DoubleRow over 4 chunks of 256
                pst = []
                for k in range(K):
                    ps = sp.tile([TP, 512], f32, name=f"split{bt}_{k}", tag="split")
                    for c in range(NC2):
                        nc.tensor.matmul(
                            ps[:],
                            w8s[k * NC2 + c][:],
                            x8_sb[:, bt, 2 * c : 2 * c + 2, :],
                            start=(c == 0),
                            stop=(c == NC2 - 1),
                            perf_mode=DR,
                        )
                    pst.append(ps)
                s = ep.tile([T, 512], f32, name=f"s_{bt}", tag="s")
                nc.scalar.activation(s[:], pss[:], AF.Identity, bias=bias_sb[:, 4:5])

                ps2 = op.tile([2, 512], f32, name=f"ps2_{bt}", tag="ps2")
                for k in range(K):
                    leaf = ep.tile([T, 512], f32, name=f"leaf{bt}_{k}", tag="leaf", bufs=4)
                    nc.scalar.activation(
                        leaf[:],
                        pst[k][:T],
                        AF.Sigmoid,
                        bias=bias_sb[:, k : k + 1],
                        scale=1.0 / WS,
                    )
                    prod = ep.tile([T, 512], f32r, name=f"prod{bt}_{k}", tag="prod", bufs=4)
                    nc.vector.tensor_mul(prod[:], leaf[:], s[:])
                    nc.tensor.matmul(
                        ps2[:],
                        g_sb[:, 32 * k : 32 * k + 2],
                        prod[:],
                        start=(k == 0),
                        stop=(k == K - 1),
                    )
                nc.scalar.activation(
                    out_sb[:, sl], ps2[:], AF.Identity, bias=fcb_sb[:]
                )

        if reps > 0:
            with tc.For_i(0, reps, 1):
                emit_body()
        else:
            emit_body()

        nc.sync.dma_start(y.ap(), out_sb[:])
    nc.compile()
    return nc


def make_in_maps(x, W1, b1, final_weight, fc_w, fc_b):
    import concourse.mybir as mybir

    np8 = mybir.dt.np(mybir.dt.float8e4)
    npb = mybir.dt.np(mybir.dt.bfloat16)

    x = np.asarray(x, np.float32)
    W1 = np.asarray(W1, np.float32)
    b1 = np.asarray(b1, np.float32)
    final_weight = np.asarray(final_weight, np.float32)
    fc_w = np.asarray(fc_w, np.float32)
    fc_b = np.asarray(fc_b, np.float32)

    xT = np.ascontiguousarray(x.T)  # [D, B]

    # w8[k*NC2+c, p, i, m] = 256*W1[m, k, 256c+128i+p] (m<100, 0-padded to 112)
    Wp = np.zeros((K, D, TP), np.float32)
    Wp[:, :, :T] = W1.transpose(1, 2, 0) * WS
    w8_host = np.ascontiguousarray(
        Wp.reshape(K, NC2, 2, 128, TP).transpose(0, 1, 3, 2, 4).reshape(
            K * NC2, 128, 2, TP
        ).astype(np8)
    )
    # wsb[d, p, m] = sum_k W1[m, k, 128d+p]
    Ws = W1.sum(1)  # [T, D]
    wsb_host = np.ascontiguousarray(Ws.T.reshape(ND, 128, T).astype(npb))

    consts_mat = np.zeros((T, 140), np.float32)
    consts_mat[:, :K] = b1
    consts_mat[:, K] = b1.sum(1)
    for k in range(K):
        for j in range(2):
            consts_mat[:, 8 + 32 * k + j] = fc_w[j, k] * final_weight
    consts_mat[0:2, 136] = fc_b

    def tilemajor(a):
        # [D, BL] -> [128, NBT, ND, 512]: partition p holds d = 128*chunk + p
        return np.ascontiguousarray(
            a.reshape(ND, 128, NBT, 512).transpose(1, 2, 0, 3)
        )

    in_maps = []
    for c in range(N_CORES):
        xc = xT[:, c * BL : (c + 1) * BL]  # [D, BL]
        in_maps.append(
            {
                "x8": tilemajor(xc.astype(np8)),
                "xb": tilemajor(xc.astype(npb)),
                "w8": w8_host,
                "wsb": wsb_host,
                "consts": consts_mat,
            }
        )
    return in_maps


def kernel(x, W1, b1, final_weight, fc_w, fc_b):
    from concourse.bass_utils import run_bass_kernel_spmd

    if "nc" not in _cache:
        _cache["nc"] = build()
    nc = _cache["nc"]
    in_maps = make_in_maps(x, W1, b1, final_weight, fc_w, fc_b)

    last_err = None
    for attempt in range(3):
        try:
            res = run_bass_kernel_spmd(nc, in_maps, core_ids=list(range(N_CORES)))
            break
        except Exception as e:  # transient device wedge: wait for recovery
            last_err = e
            time.sleep(90)
    else:
        raise last_err

    out = np.empty((B, 2), np.float32)
    for c in range(N_CORES):
        out[c * BL : (c + 1) * BL, :] = res.results[c]["y"].T
    return out


# revision 3
# speedup vs baseline: 1.0070x; 1.0070x over previous
"""DifferentiableXGB forward on 8 TRN2 NeuronCores — v11 (2x-unrolled rep loop).

Baseline (69us) compute, unchanged op-for-op; only the x data path is
restructured: tile-major DRAM layouts so each (tensor, batch-tile) is ONE
DMA of 128 contiguous runs (16 issues/rep instead of 64 — each dma_start
costs ~650ns serialized on SP+HWDGE).

Per-core device program (identical math to baseline):
  split_k^T [112t, b] = x8 . W8_k      (fp8 DoubleRow, 4 passes of 256)
  s^T [100t, b]       = xb . Ws        (bf16 exact path)
  leaf_k^T = sigmoid(split_k^T/256 + b1[:,k])   (ACT)
  prod_k^T = leaf_k^T * s^T                      (DVE, f32r out)
  y^T [2, b] = sum_k g_k^T prod_k^T + fc_b       (PE accum + ACT bias)
"""
import time
import numpy as np
from contextlib import ExitStack

N_CORES = 8
B, D, T, K = 32768, 1024, 100, 4
BL = B // N_CORES  # batch rows per core
NBT = BL // 512    # 512-wide batch tiles per core
ND = D // 128      # 128-deep contraction chunks
NC2 = D // 256     # 256-deep DoubleRow chunks
TP = 112           # tree channels padded (step%16==0 for DoubleRow lhsT)
WS = 256.0         # fp8 weight scale

_cache = {}


def build_pools(reps, pools):
    global _POOLS
    _POOLS = pools
    try:
        return build(reps)
    finally:
        _POOLS = (6, 1, 1)


_POOLS = (6, 1, 1)


def build(reps=0):
    """Build + compile the per-core Bass program. reps>0 wraps the compute
    body in a HW loop executing it `reps` times (for steady-state timing)."""
    from concourse import bacc
    import concourse.mybir as mybir
    import concourse.tile as tile

    f32, f32r = mybir.dt.float32, mybir.dt.float32r
    bf16, f8 = mybir.dt.bfloat16, mybir.dt.float8e4
    AF = mybir.ActivationFunctionType
    DR = mybir.MatmulPerfMode.DoubleRow

    nc = bacc.Bacc("TRN2", target_bir_lowering=False, debug=False)
    x8 = nc.dram_tensor("x8", [128, NBT, ND, 512], f8, kind="ExternalInput")
    xb = nc.dram_tensor("xb", [128, NBT, ND, 512], bf16, kind="ExternalInput")
    w8 = nc.dram_tensor("w8", [K * NC2, 128, 2, TP], f8, kind="ExternalInput")
    wsb = nc.dram_tensor("wsb", [ND, 128, T], bf16, kind="ExternalInput")
    # bias [0:8] | g 4 blocks of 32 [8:136] | fcb col 136 — one DMA: separate
    # small const DMAs were observed to corrupt unrelated SBUF loads
    consts = nc.dram_tensor("consts", [T, 140], f32r, kind="ExternalInput")
    y = nc.dram_tensor("y", [2, BL], f32, kind="ExternalOutput")

    with ExitStack() as ctx:
        tc = ctx.enter_context(tile.TileContext(nc))
        cp = ctx.enter_context(tc.tile_pool(name="const", bufs=1))
        wp = ctx.enter_context(tc.tile_pool(name="wp", bufs=1))
        xp = ctx.enter_context(tc.tile_pool(name="xp", bufs=1))
        ep = ctx.enter_context(tc.tile_pool(name="ep", bufs=3))
        # sp=6: split psum slots span two tiles of slack so a tile's matmuls
        # never serialize behind the previous tile's sigmoid reads
        sp = ctx.enter_context(tc.tile_pool(name="sp", bufs=_POOLS[0], space="PSUM"))
        ssp = ctx.enter_context(tc.tile_pool(name="ssp", bufs=_POOLS[1], space="PSUM"))
        op = ctx.enter_context(tc.tile_pool(name="op", bufs=_POOLS[2], space="PSUM"))

        consts_sb = cp.tile([T, 140], f32r, name="consts_sb", tag="consts_sb")
        nc.sync.dma_start(consts_sb[:], consts.ap())
        bias_sb = consts_sb[:, 0:8].bitcast(f32)
        g_sb = consts_sb[:, 8:136]
        fcb_sb = consts_sb[0:2, 136:137].bitcast(f32)
        out_sb = cp.tile([2, BL], f32, name="out_sb", tag="out_sb")

        w8s = []
        for i in range(K * NC2):
            wt = wp.tile([128, 2, TP], f8, name=f"w8_{i}", tag=f"w8_{i}")
            nc.sync.dma_start(wt[:], w8.ap()[i])
            w8s.append(wt)
        wss = []
        for d in range(ND):
            wt = wp.tile([128, T], bf16, name=f"ws{d}", tag=f"ws{d}")
            nc.sync.dma_start(wt[:], wsb.ap()[d])
            wss.append(wt)

        x8_sb = xp.tile([128, NBT, ND, 512], f8, name="x8_sb", tag="x8_sb")
        xb_sb = xp.tile([128, NBT * ND, 512], bf16, name="xb_sb", tag="xb_sb")

        x8ap, xbap = x8.ap(), xb.ap()

        def emit_body():
            # one DMA per (tensor, tile): 128 partitions x 4/8KB contiguous
            for bt in range(NBT):
                nc.sync.dma_start(x8_sb[:, bt], x8ap[:, bt])
                nc.sync.dma_start(xb_sb[:, bt * ND : (bt + 1) * ND, :], xbap[:, bt])
            for bt in range(NBT):
                sl = slice(bt * 512, (bt + 1) * 512)
                # exact-s group, bf16 over 8 chunks of 128
                pss = ssp.tile([T, 512], f32, name=f"ss{bt}", tag="ss")
                for d in range(ND):
                    nc.tensor.matmul(
                        pss[:],
                        wss[d][:],
                        xb_sb[:, bt * ND + d, :],
                        start=(d == 0),
                        stop=(d == ND - 1),
                    )
                # 4 split groups, fp8 DoubleRow over 4 chunks of 256
                pst = []
                for k in range(K):
                    ps = sp.tile([TP, 512], f32, name=f"split{bt}_{k}", tag="split")
                    for c in range(NC2):
                        nc.tensor.matmul(
                            ps[:],
                            w8s[k * NC2 + c][:],
                            x8_sb[:, bt, 2 * c : 2 * c + 2, :],
                            start=(c == 0),
                            stop=(c == NC2 - 1),
                            perf_mode=DR,
                        )
                    pst.append(ps)
                s = ep.tile([T, 512], f32, name=f"s_{bt}", tag="s")
                nc.scalar.activation(s[:], pss[:], AF.Identity, bias=bias_sb[:, 4:5])

                ps2 = op.tile([2, 512], f32, name=f"ps2_{bt}", tag="ps2")
                for k in range(K):
                    leaf = ep.tile([T, 512], f32, name=f"leaf{bt}_{k}", tag="leaf", bufs=4)
                    nc.scalar.activation(
                        leaf[:],
                        pst[k][:T],
                        AF.Sigmoid,
                        bias=bias_sb[:, k : k + 1],
                        scale=1.0 / WS,
                    )
                    prod = ep.tile([T, 512], f32r, name=f"prod{bt}_{k}", tag="prod", bufs=4)
                    nc.vector.tensor_mul(prod[:], leaf[:], s[:])
                    nc.tensor.matmul(
                        ps2[:],
                        g_sb[:, 32 * k : 32 * k + 2],
                        prod[:],
                        start=(k == 0),
                        stop=(k == K - 1),
                    )
                nc.scalar.activation(
                    out_sb[:, sl], ps2[:], AF.Identity, bias=fcb_sb[:]
                )

        # 2x-unrolled hardware loop: halves any per-iteration back-edge
        # rendezvous cost. Handles odd reps exactly (loop runs reps//2
        # double-bodies plus one standalone body).
        if reps > 0:
            if reps >= 2:
                with tc.For_i(0, reps // 2, 1):
                    emit_body()
                    emit_body()
            if reps % 2:
                emit_body()
        else:
            emit_body()

        nc.sync.dma_start(y.ap(), out_sb[:])
    nc.compile()
    return nc


def make_in_maps(x, W1, b1, final_weight, fc_w, fc_b):
    import concourse.mybir as mybir

    np8 = mybir.dt.np(mybir.dt.float8e4)
    npb = mybir.dt.np(mybir.dt.bfloat16)

    x = np.asarray(x, np.float32)
    W1 = np.asarray(W1, np.float32)
    b1 = np.asarray(b1, np.float32)
    final_weight = np.asarray(final_weight, np.float32)
    fc_w = np.asarray(fc_w, np.float32)
    fc_b = np.asarray(fc_b, np.float32)

    xT = np.ascontiguousarray(x.T)  # [D, B]

    # w8[k*NC2+c, p, i, m] = 256*W1[m, k, 256c+128i+p] (m<100, 0-padded to 112)
    Wp = np.zeros((K, D, TP), np.float32)
    Wp[:, :, :T] = W1.transpose(1, 2, 0) * WS
    w8_host = np.ascontiguousarray(
        Wp.reshape(K, NC2, 2, 128, TP).transpose(0, 1, 3, 2, 4).reshape(
            K * NC2, 128, 2, TP
        ).astype(np8)
    )
    # wsb[d, p, m] = sum_k W1[m, k, 128d+p]
    Ws = W1.sum(1)  # [T, D]
    wsb_host = np.ascontiguousarray(Ws.T.reshape(ND, 128, T).astype(npb))

    consts_mat = np.zeros((T, 140), np.float32)
    consts_mat[:, :K] = b1
    consts_mat[:, K] = b1.sum(1)
    for k in range(K):
        for j in range(2):
            consts_mat[:, 8 + 32 * k + j] = fc_w[j, k] * final_weight
    consts_mat[0:2, 136] = fc_b

    def tilemajor(a):
        # [D, BL] -> [128, NBT, ND, 512]: partition p holds d = 128*chunk + p
        return np.ascontiguousarray(
            a.reshape(ND, 128, NBT, 512).transpose(1, 2, 0, 3)
        )

    in_maps = []
    for c in range(N_CORES):
        xc = xT[:, c * BL : (c + 1) * BL]  # [D, BL]
        in_maps.append(
            {
                "x8": tilemajor(xc.astype(np8)),
                "xb": tilemajor(xc.astype(npb)),
                "w8": w8_host,
                "wsb": wsb_host,
                "consts": consts_mat,
            }
        )
    return in_maps


def kernel(x, W1, b1, final_weight, fc_w, fc_b):
    from concourse.bass_utils import run_bass_kernel_spmd

    if "nc" not in _cache:
        _cache["nc"] = build()
    nc = _cache["nc"]
    in_maps = make_in_maps(x, W1, b1, final_weight, fc_w, fc_b)

    last_err = None
    for attempt in range(3):
        try:
            res = run_bass_kernel_spmd(nc, in_maps, core_ids=list(range(N_CORES)))
            break
        except Exception as e:  # transient device wedge: wait for recovery
            last_err = e
            time.sleep(90)
    else:
        raise last_err

    out = np.empty((B, 2), np.float32)
    for c in range(N_CORES):
        out[c * BL : (c + 1) * BL, :] = res.results[c]["y"].T
    return out
